# revision 51
# baseline (speedup 1.0000x reference)
"""Causal self-attention (B=2, T=2048, C=1024, nh=16) on 8 TRN2 NeuronCores.

Sharding: core c = 4*b + g handles batch b (2048 tokens) and head-group g
(4 heads).  Megatron-style: QKV rows and proj columns sharded by head group;
the proj partial sums are reduced on the host (the "all-reduce").

Per-core kernel, all matmul operands bf16 (same PE rate as fp32r but FWL
weight loads + half SBUF/DMA bandwidth), PSUM accumulation fp32:
  1. QKV projection kqvT[f,t] = Wl @ x_b.T -> ktp (k, head-pair packed on
     partitions: even head 0:64, odd head 64:128), q and v slots in kqv.
  2. v tiles PE-transposed to [s,d] layout, packed next to shared all-ones
     column blocks so the PV matmul also emits softmax row-sums broadcast
     across the complement 64 partitions.
  3. QK row-tiled: per j-tile the two heads of a pair run CONCURRENTLY in
     PE row groups (0,0)/(64,0) with 64-partition operands (no zero pad).
     exp on ACT -> pt bf16 (parked in SBUF), multiplicative 0/1 mask on
     diagonal blocks (gpsimd), PV accumulation v_aug.T @ P.T -> yT+rowsum.
  4. Software pipeline: the QK+exp stream runs LOOKAHEAD groups ahead of
     the PV stream so ACT's quadratic exp load is prepaid early; QKV(n+1),
     v transposes and proj(n-1) matmuls braid into PE slack.
  5. normalize: fast-approx reciprocal off PSUM rowsums + DMA partition
     shift; multiplies read yT straight from PSUM. proj partials on PE,
     bf16 partial outputs summed on host.
"""

import os
import numpy as np

B, T, C, NH, HD = 2, 2048, 1024, 16, 64
HPC = 4  # heads per core
NCORES = 8
NCH = 4       # 512-wide t-chunks
CHW = 512
LOOKAHEAD = 12

_cache = {}


def _build_nc():
    from contextlib import ExitStack

    import concourse.bass as bass
    import concourse.tile as tile
    from concourse import bacc, mybir

    f32 = mybir.dt.float32
    bf16 = mybir.dt.bfloat16
    f8 = mybir.dt.float8e4
    AF = mybir.ActivationFunctionType
    OP = mybir.AluOpType

    nc = bacc.Bacc("TRN2", target_bir_lowering=False, debug=False,
                   num_devices=NCORES)

    xt = nc.dram_tensor("xt", [C, T], bf16, kind="ExternalInput").ap()
    wkqv = nc.dram_tensor("wkqv", [C, 3 * HPC * HD], bf16,
                          kind="ExternalInput").ap()
    bkq = nc.dram_tensor("bkq", [128, 6], f32, kind="ExternalInput").ap()
    wproj = nc.dram_tensor("wproj", [HPC * HD, C], bf16,
                           kind="ExternalInput").ap()
    bp = nc.dram_tensor("bp", [128, 8], f32, kind="ExternalInput").ap()
    amask_d = nc.dram_tensor("amask", [128, 128], bf16,
                             kind="ExternalInput").ap()
    ident_d = nc.dram_tensor("ident", [128, 128], bf16,
                             kind="ExternalInput").ap()
    outp = nc.dram_tensor("outp", [C, T], bf16, kind="ExternalOutput").ap()

    with tile.TileContext(nc) as tc, ExitStack() as ctx:
        sing = ctx.enter_context(tc.tile_pool(name="sing", bufs=1))
        xpool = ctx.enter_context(tc.tile_pool(name="xpool", bufs=2))
        ptp = ctx.enter_context(tc.tile_pool(name="ptp",
                                             bufs=2 * (LOOKAHEAD + 2)))
        rbp = ctx.enter_context(tc.tile_pool(name="rbp", bufs=2))
        osp = ctx.enter_context(tc.tile_pool(name="osp", bufs=3))
        ps = ctx.enter_context(tc.tile_pool(name="ps", bufs=2, space="PSUM"))

        # ---- resident SBUF tensors ----
        wk = sing.tile([128, 8, 768], bf16, name="wk")
        ktp = sing.tile([128, 2, T], bf16, name="ktp")
        kqv = sing.tile([128, 4, T], bf16, name="kqv")
        vsb = sing.tile([128, 32, 256], bf16, name="vsb")
        ysb = sing.tile([128, 2, T], bf16, name="ysb")
        wp = sing.tile([128, 2, C], bf16, name="wp")
        bkq_s = sing.tile([128, 6], f32, name="bkq_s")
        bp_s = sing.tile([128, 8], f32, name="bp_s")
        amask = sing.tile([128, 128], bf16, name="amask")
        ident = sing.tile([128, 128], bf16, name="ident")

        nc.vector.memset(vsb[:, :, 64:192], 1.0)

        xt_r = xt.rearrange("(kk p) t -> p kk t", p=128)
        wkqv_r = wkqv.rearrange("(kk p) f -> p kk f", p=128)

        xts_tiles = {}

        def prefetch_x(n):
            nxt = xpool.tile([128, 8, CHW], bf16, name="xts")
            for k in range(8):
                eng = nc.scalar if k % 2 else nc.sync
                eng.dma_start(nxt[:, k, :], xt_r[:, k, n * CHW:(n + 1) * CHW])
            xts_tiles[n] = nxt

        # initial loads: interleave weights and first x chunk per k-tile so
        # the first QKV matmuls start as soon as possible
        xts0 = xpool.tile([128, 8, CHW], bf16, name="xts")
        xts_tiles[0] = xts0
        for k in range(8):
            e1, e2 = (nc.scalar, nc.sync) if k % 2 else (nc.sync, nc.scalar)
            e1.dma_start(wk[:, k, :], wkqv_r[:, k, :])
            e2.dma_start(xts0[:, k, :], xt_r[:, k, 0:CHW])
        nc.sync.dma_start(bkq_s, bkq)
        nc.scalar.dma_start(ident, ident_d)
        nc.scalar.dma_start(amask, amask_d)
        nc.sync.dma_start(wp, wproj.rearrange("(kk p) f -> p kk f", p=128))
        nc.sync.dma_start(bp_s, bp)

        def v_stationary(j, hf, sl):
            off = (j * 2 + hf) * 256 + 128 * sl
            return vsb.rearrange("p a b -> p (a b)")[:, off:off + 128]

        def qkv_step(n, m):
            """one full m-tile: 8 contraction matmuls + drain (atomic so the
            shared acc tag never holds a live accumulator across items)"""
            cols = slice(n * CHW, (n + 1) * CHW)
            acc = ps.tile([128, CHW], f32, name="acc", tag="acc", bufs=2)
            for k in range(8):
                nc.tensor.matmul(acc, wk[:, k, m * 128:(m + 1) * 128],
                                 xts_tiles[n][:, k, :],
                                 start=(k == 0), stop=(k == 7),
                                 skip_group_check=True)
            dst = ktp[:, m, cols] if m < 2 else kqv[:, m - 2, cols]
            nc.vector.tensor_scalar_add(out=dst, in0=acc,
                                        scalar1=bkq_s[:, m:m + 1])

        def v_transpose_step(j, hf):
            tpf = ps.tile([128, CHW], f32, name="tp", tag="acc", bufs=2)
            tp = tpf.bitcast(bf16)[:, 0:128]
            nc.tensor.transpose(tp, kqv[:, 2 + hf, j * 128:(j + 1) * 128],
                                ident)
            base = vsb.rearrange("p a b -> p (a b)")
            anch = base[:, (j * 2 + hf) * 256:(j * 2 + hf) * 256 + 1]
            dst = bass.AP(tensor=anch.tensor, offset=anch.offset,
                          ap=[anch.ap[0], [192, 2], [1, 64]])
            nc.vector.tensor_copy(dst, tp.rearrange("p (a b) -> p a b", a=2))

        def proj_step(pn, o):
            acc = ps.tile([128, CHW], f32, name="pacc", tag="acc", bufs=2)
            for kk in range(2):
                nc.tensor.matmul(acc, wp[:, kk, o * 128:(o + 1) * 128],
                                 ysb[:, kk, pn * CHW:(pn + 1) * CHW],
                                 start=(kk == 0), stop=(kk == 1),
                                 skip_group_check=True)
            ot = osp.tile([128, CHW], bf16, name="ot")
            nc.vector.tensor_scalar_add(out=ot, in0=acc,
                                        scalar1=bp_s[:, o:o + 1])
            nc.sync.dma_start(
                outp[o * 128:(o + 1) * 128, pn * CHW:(pn + 1) * CHW], ot)

        # ---------------- software pipeline ----------------
        groups = [(n, hf, a)
                  for n in range(NCH) for hf in range(2)
                  for a in range(2 * n + 2)]
        NG = len(groups)

        # filler queue with named barriers: ("mark", name) entries
        fillers = []
        marks_done = set()

        def drain_until(name):
            while name not in marks_done:
                item = fillers.pop(0)
                if isinstance(item, tuple):
                    marks_done.add(item[1])
                else:
                    item()

        def consume(k):
            while k > 0 and fillers:
                item = fillers.pop(0)
                if isinstance(item, tuple):
                    marks_done.add(item[1])
                else:
                    item()
                    k -= 1

        def add_qk_chunk_fillers(n):
            """called when the QK stream enters chunk n: queue next chunk's
            QKV matmuls and v transposes"""
            if n + 1 < NCH:
                fillers.append(lambda m=n + 1: prefetch_x(m))
                for m in range(6):
                    fillers.append(lambda m=m, nn=n + 1: qkv_step(nn, m))
                fillers.append(("mark", f"qkv{n + 1}"))
                for j in range(4 * (n + 1), 4 * (n + 1) + 4):
                    for hf in range(2):
                        fillers.append(
                            lambda j=j, hf=hf: v_transpose_step(j, hf))
                fillers.append(("mark", f"vt{n + 1}"))

        pts = {}

        def qk_emit(g):
            n, hf, a = groups[g]
            if a == 0 and hf == 0:
                drain_until(f"qkv{n}")  # ktp/q for chunk n must be emitted
                add_qk_chunk_fillers(n)
            c0s = [max(0, 128 * (2 * a + idx) - 512 * n) for idx in range(2)]
            sss = [ps.tile([128, 2, CHW], f32, name=f"ss{sl}", tag="ss",
                           bufs=2) for sl in range(2)]
            for idx in range(2):
                j = 2 * a + idx
                c0 = c0s[idx]
                for sl in range(2):
                    p0, p1 = 64 * sl, 64 * (sl + 1)
                    nc.tensor.matmul(
                        sss[sl][:, idx, c0:],
                        ktp[p0:p1, hf, j * 128:(j + 1) * 128],
                        kqv[p0:p1, hf, n * CHW + c0:(n + 1) * CHW],
                        start=True, stop=True, skip_group_check=True)
            cur = []
            for sl in range(2):
                pt = ptp.tile([128, 2, CHW], bf16, name="pt")
                cur.append(pt)
                if c0s[0] == 0 and c0s[1] == 0:
                    nc.scalar.activation(out=pt, in_=sss[sl], func=AF.Exp)
                else:
                    for idx in range(2):
                        c0 = c0s[idx]
                        nc.scalar.activation(out=pt[:, idx, c0:],
                                             in_=sss[sl][:, idx, c0:],
                                             func=AF.Exp)
                for idx in range(2):
                    j = 2 * a + idx
                    if j >= 4 * n:
                        c0 = c0s[idx]
                        nc.gpsimd.tensor_tensor(
                            out=pt[:, idx, c0:c0 + 128],
                            in0=pt[:, idx, c0:c0 + 128],
                            in1=amask, op=OP.mult)
            pts[g] = cur

        pys_cur = [None]
        tail_accs = []

        def pv_emit(g):
            n, hf, a = groups[g]
            jmax = 4 * n + 3
            cols = slice(n * CHW, (n + 1) * CHW)
            if hf == 0 and a == 0 and n >= 1:
                for o in range(8):
                    fillers.append(lambda pn=n - 1, o=o: proj_step(pn, o))
            if a == 0:
                pys_cur[0] = [ps.tile([128, CHW], f32, name=f"py{sl}",
                                      tag="py", bufs=2) for sl in range(2)]
            pys = pys_cur[0]
            if 2 * a + 1 >= 4 * n:
                drain_until(f"vt{n}")  # this group's v blocks must be placed
            c0s = [max(0, 128 * (2 * a + idx) - 512 * n) for idx in range(2)]
            for sl in range(2):
                for idx in range(2):
                    j = 2 * a + idx
                    c0 = c0s[idx]
                    nc.tensor.matmul(
                        pys[sl][:, c0:], v_stationary(j, hf, sl),
                        pts[g][sl][:, idx, c0:],
                        start=(j == 0), stop=(j == jmax),
                        skip_group_check=True)
            del pts[g]
            if n == 3 and hf == 0 and a == 2 * n + 1:
                # after this norm, ysb[:,0,chunk3] is final: braid the first
                # two tail-proj blocks' kk=0 accumulations into the pipeline
                def tail_kk0(op2):
                    acc = ps.tile([128, 2, CHW], f32, name="tacc", tag="ss",
                                  bufs=2)
                    for i in range(2):
                        o = 2 * op2 + i
                        nc.tensor.matmul(
                            acc[:, i, :], wp[:, 0, o * 128:(o + 1) * 128],
                            ysb[:, 0, 3 * CHW:4 * CHW],
                            start=True, stop=False, skip_group_check=True)
                    tail_accs.append(acc)
                fillers.append(lambda: tail_kk0(0))
                fillers.append(lambda: tail_kk0(1))
            if a == 2 * n + 1:
                # ---- normalization for the head pair ----
                ri = rbp.tile([128, CHW], f32, name="ri")
                rb = rbp.tile([128, CHW], f32, name="rb")
                nc.vector.tensor_copy(ri[64:128, :], pys[0][64:128, :])
                nc.sync.dma_start(rb[0:64, :], ri[64:128, :])
                nc.vector.reciprocal_approx_fast(ri[0:64, :],
                                                 pys[1][0:64, :])
                nc.vector.reciprocal_approx_fast(rb[0:64, :], rb[0:64, :])
                nc.vector.tensor_tensor(out=ysb[0:64, hf, cols],
                                        in0=pys[0][0:64, :],
                                        in1=rb[0:64, :], op=OP.mult)
                nc.sync.dma_start(rb[64:128, :], ri[0:64, :])
                nc.vector.tensor_tensor(out=ysb[64:128, hf, cols],
                                        in0=pys[1][64:128, :],
                                        in1=rb[64:128, :], op=OP.mult)

        # ---- chunk-0 QKV k/q tiles, then the pipeline ----
        with nc.named_scope("qkv0"):
            for m in range(4):
                qkv_step(0, m)
        # warmup fillers: v tiles of chunk 0 (k/q already emitted above)
        fillers.append(("mark", "qkv0"))
        for m in (4, 5):
            fillers.append(lambda m=m: qkv_step(0, m))
        for j in range(4):
            for hf in range(2):
                fillers.append(lambda j=j, hf=hf: v_transpose_step(j, hf))
        fillers.append(("mark", "vt0"))

        HORIZON = 6
        with nc.named_scope("pipe"):
            for slot in range(NG + LOOKAHEAD):
                if slot >= LOOKAHEAD:
                    pv_emit(slot - LOOKAHEAD)
                nitems = sum(1 for f in fillers if not isinstance(f, tuple))
                rem = min(NG + LOOKAHEAD - slot, HORIZON)
                consume(max(0, (nitems + rem - 1) // rem))
                if slot < NG:
                    qk_emit(slot)
            while fillers:
                consume(1)

        # ---- tail: proj for chunk 3 in ss-shaped o-pairs ----
        with nc.named_scope("tail"):
            for op2 in range(4):
                if op2 < len(tail_accs):  # kk=0 already braided
                    acc = tail_accs[op2]
                    for i in range(2):
                        o = 2 * op2 + i
                        nc.tensor.matmul(
                            acc[:, i, :], wp[:, 1, o * 128:(o + 1) * 128],
                            ysb[:, 1, 3 * CHW:4 * CHW],
                            start=False, stop=True, skip_group_check=True)
                else:
                    acc = ps.tile([128, 2, CHW], f32, name="tacc", tag="ss",
                                  bufs=2)
                    for i in range(2):
                        o = 2 * op2 + i
                        for kk in range(2):
                            nc.tensor.matmul(
                                acc[:, i, :],
                                wp[:, kk, o * 128:(o + 1) * 128],
                                ysb[:, kk, 3 * CHW:4 * CHW],
                                start=(kk == 0), stop=(kk == 1),
                                skip_group_check=True)
                ot = osp.tile([128, 2, CHW], bf16, name="ot2")
                for i in range(2):
                    o = 2 * op2 + i
                    if i == 0:
                        nc.vector.tensor_scalar_add(out=ot[:, i, :],
                                                    in0=acc[:, i, :],
                                                    scalar1=bp_s[:, o:o + 1])
                    else:  # ACT is idle at the tail — split the drains
                        nc.scalar.activation(out=ot[:, i, :],
                                             in_=acc[:, i, :],
                                             func=AF.Identity,
                                             bias=bp_s[:, o:o + 1])
                nc.sync.dma_start(
                    outp[op2 * 256:(op2 + 1) * 256,
                         3 * CHW:4 * CHW].rearrange("(a p) t -> p a t", a=2),
                    ot)

    nc.compile()
    return nc


def _host_inputs(x, W_kqv, b_kqv, W_proj, b_proj):
    import ml_dtypes
    bf16 = ml_dtypes.bfloat16

    x = np.asarray(x, dtype=np.float32)
    W_kqv = np.asarray(W_kqv, dtype=np.float32)
    b_kqv = np.asarray(b_kqv, dtype=np.float32)
    W_proj = np.asarray(W_proj, dtype=np.float32)
    b_proj = np.asarray(b_proj, dtype=np.float32)

    ss, tt = np.meshgrid(np.arange(128), np.arange(128), indexing="ij")
    amask = (ss <= tt).astype(bf16)  # 0/1 multiplicative mask
    ident = np.eye(128, dtype=bf16)

    xts = [np.ascontiguousarray(x[b].T.astype(bf16)) for b in range(B)]

    in_maps = []
    for c in range(NCORES):
        b, g = c // 4, c % 4
        heads = [4 * g + i for i in range(HPC)]
        wl = np.concatenate(
            [W_kqv[h * 192:h * 192 + 64] for h in heads]
            + [W_kqv[h * 192 + 64:h * 192 + 128] * 0.125 for h in heads]
            + [W_kqv[h * 192 + 128:h * 192 + 192] for h in heads], axis=0)
        bl = np.concatenate(
            [b_kqv[h * 192:h * 192 + 64] for h in heads]
            + [b_kqv[h * 192 + 64:h * 192 + 128] * 0.125 for h in heads]
            + [b_kqv[h * 192 + 128:h * 192 + 192] for h in heads])
        bpl = b_proj if g == 0 else np.zeros_like(b_proj)
        in_maps.append({
            "xt": xts[b],
            "wkqv": np.ascontiguousarray(wl.T.astype(bf16)),
            "bkq": np.ascontiguousarray(bl.reshape(6, 128).T),
            "wproj": np.ascontiguousarray(
                W_proj[:, 256 * g:256 * (g + 1)].T.astype(bf16)),
            "bp": np.ascontiguousarray(bpl.reshape(8, 128).T),
            "amask": amask,
            "ident": ident,
        })
    return in_maps


def kernel(x, W_kqv, b_kqv, W_proj, b_proj):
    from concourse.bass_utils import run_bass_kernel_spmd

    if "nc" not in _cache:
        _cache["nc"] = _build_nc()
    nc = _cache["nc"]

    in_maps = _host_inputs(x, W_kqv, b_kqv, W_proj, b_proj)
    trace = bool(int(os.environ.get("KERNEL_TRACE", "0")))
    r = run_bass_kernel_spmd(nc, in_maps, core_ids=list(range(NCORES)),
                             trace=trace)
    if trace:
        _cache["last_results"] = r
        print(f"HW exec time: {r.exec_time_ns} ns")

    out = np.empty((B, T, C), dtype=np.float32)
    for b in range(B):
        acc = np.zeros((C, T), dtype=np.float32)
        for g in range(4):
            acc += np.asarray(r.results[4 * b + g]["outp"],
                              dtype=np.float32)
        out[b] = acc.T
    return out


# revision 54
# speedup vs baseline: 1.1898x; 1.1898x over previous
"""Causal self-attention (B=2, T=2048, C=1024, nh=16) on 8 TRN2 NeuronCores.

Sharding: core c = 4*b + g handles batch b (2048 tokens) and head-group g
(4 heads).  Megatron-style: QKV rows and proj columns sharded by head group;
the proj partial sums are reduced on the host (the "all-reduce").

Per-core kernel, all matmul operands bf16 (same PE rate as fp32r but FWL
weight loads + half SBUF/DMA bandwidth), PSUM accumulation fp32:
  1. QKV projection kqvT[f,t] = Wl @ x_b.T -> ktp (k, head-pair packed on
     partitions: even head 0:64, odd head 64:128), q and v slots in kqv.
  2. v tiles PE-transposed to [s,d] layout, packed next to shared all-ones
     column blocks so the PV matmul also emits softmax row-sums broadcast
     across the complement 64 partitions.
  3. QK row-tiled: per j-tile the two heads of a pair run CONCURRENTLY in
     PE row groups (0,0)/(64,0) with 64-partition operands (no zero pad).
     exp on ACT -> pt bf16 (parked in SBUF), multiplicative 0/1 mask on
     diagonal blocks (gpsimd), PV accumulation v_aug.T @ P.T -> yT+rowsum.
  4. Software pipeline: the QK+exp stream runs LOOKAHEAD groups ahead of
     the PV stream so ACT's quadratic exp load is prepaid early; QKV(n+1),
     v transposes and proj(n-1) matmuls braid into PE slack.
  5. normalize: fast-approx reciprocal off PSUM rowsums + DMA partition
     shift; multiplies read yT straight from PSUM. proj partials on PE,
     bf16 partial outputs summed on host.
"""

import os
import numpy as np

B, T, C, NH, HD = 2, 2048, 1024, 16, 64
HPC = 4  # heads per core
NCORES = 8
NCH = 4       # 512-wide t-chunks
CHW = 512
LOOKAHEAD = 12

_cache = {}


def _build_nc():
    from contextlib import ExitStack

    import concourse.bass as bass
    import concourse.tile as tile
    from concourse import bacc, mybir

    f32 = mybir.dt.float32
    bf16 = mybir.dt.bfloat16
    f8 = mybir.dt.float8e4
    AF = mybir.ActivationFunctionType
    OP = mybir.AluOpType

    nc = bacc.Bacc("TRN2", target_bir_lowering=False, debug=False,
                   num_devices=NCORES)

    xt = nc.dram_tensor("xt", [C, T], bf16, kind="ExternalInput").ap()
    wkqv = nc.dram_tensor("wkqv", [C, 3 * HPC * HD], bf16,
                          kind="ExternalInput").ap()
    bkq = nc.dram_tensor("bkq", [128, 6], f32, kind="ExternalInput").ap()
    wproj = nc.dram_tensor("wproj", [HPC * HD, C], bf16,
                           kind="ExternalInput").ap()
    bp = nc.dram_tensor("bp", [128, 8], f32, kind="ExternalInput").ap()
    amask_d = nc.dram_tensor("amask", [128, 128], bf16,
                             kind="ExternalInput").ap()
    ident_d = nc.dram_tensor("ident", [128, 128], bf16,
                             kind="ExternalInput").ap()
    outp = nc.dram_tensor("outp", [C, T], bf16, kind="ExternalOutput").ap()

    with tile.TileContext(nc) as tc, ExitStack() as ctx:
        sing = ctx.enter_context(tc.tile_pool(name="sing", bufs=1))
        xpool = ctx.enter_context(tc.tile_pool(name="xpool", bufs=2))
        ptp = ctx.enter_context(tc.tile_pool(name="ptp",
                                             bufs=2 * (LOOKAHEAD + 2)))
        rbp = ctx.enter_context(tc.tile_pool(name="rbp", bufs=2))
        osp = ctx.enter_context(tc.tile_pool(name="osp", bufs=3))
        ps = ctx.enter_context(tc.tile_pool(name="ps", bufs=2, space="PSUM"))

        # ---- resident SBUF tensors ----
        wk = sing.tile([128, 8, 768], bf16, name="wk")
        ktp = sing.tile([128, 2, T], bf16, name="ktp")
        kqv = sing.tile([128, 4, T], bf16, name="kqv")
        vsb = sing.tile([128, 32, 256], bf16, name="vsb")
        ysb = sing.tile([128, 2, T], bf16, name="ysb")
        wp = sing.tile([128, 2, C], bf16, name="wp")
        bkq_s = sing.tile([128, 6], f32, name="bkq_s")
        bp_s = sing.tile([128, 8], f32, name="bp_s")
        amask = sing.tile([128, 128], bf16, name="amask")
        ident = sing.tile([128, 128], bf16, name="ident")

        nc.vector.memset(vsb[:, :, 64:192], 1.0)

        xt_r = xt.rearrange("(kk p) t -> p kk t", p=128)
        wkqv_r = wkqv.rearrange("(kk p) f -> p kk f", p=128)

        xts_tiles = {}

        def prefetch_x(n):
            nxt = xpool.tile([128, 8, CHW], bf16, name="xts")
            for k in range(8):
                eng = nc.scalar if k % 2 else nc.sync
                eng.dma_start(nxt[:, k, :], xt_r[:, k, n * CHW:(n + 1) * CHW])
            xts_tiles[n] = nxt

        # initial loads: interleave weights and first x chunk per k-tile so
        # the first QKV matmuls start as soon as possible
        xts0 = xpool.tile([128, 8, CHW], bf16, name="xts")
        xts_tiles[0] = xts0
        for k in range(8):
            e1, e2 = (nc.scalar, nc.sync) if k % 2 else (nc.sync, nc.scalar)
            e1.dma_start(wk[:, k, :], wkqv_r[:, k, :])
            e2.dma_start(xts0[:, k, :], xt_r[:, k, 0:CHW])
        nc.sync.dma_start(bkq_s, bkq)
        nc.scalar.dma_start(ident, ident_d)
        nc.scalar.dma_start(amask, amask_d)
        nc.sync.dma_start(wp, wproj.rearrange("(kk p) f -> p kk f", p=128))
        nc.sync.dma_start(bp_s, bp)

        def v_stationary(j, hf, sl):
            off = (j * 2 + hf) * 256 + 128 * sl
            return vsb.rearrange("p a b -> p (a b)")[:, off:off + 128]

        def qkv_step(n, m):
            """one full m-tile: 8 contraction matmuls + drain (atomic so the
            shared acc tag never holds a live accumulator across items)"""
            cols = slice(n * CHW, (n + 1) * CHW)
            acc = ps.tile([128, CHW], f32, name="acc", tag="acc", bufs=2)
            for k in range(8):
                nc.tensor.matmul(acc, wk[:, k, m * 128:(m + 1) * 128],
                                 xts_tiles[n][:, k, :],
                                 start=(k == 0), stop=(k == 7),
                                 skip_group_check=True)
            dst = ktp[:, m, cols] if m < 2 else kqv[:, m - 2, cols]
            nc.vector.tensor_scalar_add(out=dst, in0=acc,
                                        scalar1=bkq_s[:, m:m + 1])

        def v_transpose_step(j, hf):
            tpf = ps.tile([128, CHW], f32, name="tp", tag="acc", bufs=2)
            tp = tpf.bitcast(bf16)[:, 0:128]
            nc.tensor.transpose(tp, kqv[:, 2 + hf, j * 128:(j + 1) * 128],
                                ident)
            base = vsb.rearrange("p a b -> p (a b)")
            anch = base[:, (j * 2 + hf) * 256:(j * 2 + hf) * 256 + 1]
            dst = bass.AP(tensor=anch.tensor, offset=anch.offset,
                          ap=[anch.ap[0], [192, 2], [1, 64]])
            nc.vector.tensor_copy(dst, tp.rearrange("p (a b) -> p a b", a=2))

        def proj_step(pn, o):
            acc = ps.tile([128, CHW], f32, name="pacc", tag="acc", bufs=2)
            for kk in range(2):
                nc.tensor.matmul(acc, wp[:, kk, o * 128:(o + 1) * 128],
                                 ysb[:, kk, pn * CHW:(pn + 1) * CHW],
                                 start=(kk == 0), stop=(kk == 1),
                                 skip_group_check=True)
            ot = osp.tile([128, CHW], bf16, name="ot")
            nc.vector.tensor_scalar_add(out=ot, in0=acc,
                                        scalar1=bp_s[:, o:o + 1])
            nc.sync.dma_start(
                outp[o * 128:(o + 1) * 128, pn * CHW:(pn + 1) * CHW], ot)

        # ---------------- software pipeline ----------------
        groups = [(n, hf, a)
                  for n in range(NCH) for hf in range(2)
                  for a in range(2 * n + 2)]
        NG = len(groups)

        # filler queue with named barriers: ("mark", name) entries
        fillers = []
        marks_done = set()

        def drain_until(name):
            while name not in marks_done:
                item = fillers.pop(0)
                if isinstance(item, tuple):
                    marks_done.add(item[1])
                else:
                    item()

        def consume(k):
            while k > 0 and fillers:
                item = fillers.pop(0)
                if isinstance(item, tuple):
                    marks_done.add(item[1])
                else:
                    item()
                    k -= 1

        def add_qk_chunk_fillers(n):
            """called when the QK stream enters chunk n: queue next chunk's
            QKV matmuls and v transposes"""
            if n + 1 < NCH:
                fillers.append(lambda m=n + 1: prefetch_x(m))
                for m in range(6):
                    fillers.append(lambda m=m, nn=n + 1: qkv_step(nn, m))
                fillers.append(("mark", f"qkv{n + 1}"))
                for j in range(4 * (n + 1), 4 * (n + 1) + 4):
                    for hf in range(2):
                        fillers.append(
                            lambda j=j, hf=hf: v_transpose_step(j, hf))
                fillers.append(("mark", f"vt{n + 1}"))

        pts = {}

        def qk_emit(g):
            n, hf, a = groups[g]
            if a == 0 and hf == 0:
                drain_until(f"qkv{n}")  # ktp/q for chunk n must be emitted
                add_qk_chunk_fillers(n)
            c0s = [max(0, 128 * (2 * a + idx) - 512 * n) for idx in range(2)]
            sss = [ps.tile([128, 2, CHW], f32, name=f"ss{sl}", tag="ss",
                           bufs=2) for sl in range(2)]
            for idx in range(2):
                j = 2 * a + idx
                c0 = c0s[idx]
                for sl in range(2):
                    p0, p1 = 64 * sl, 64 * (sl + 1)
                    nc.tensor.matmul(
                        sss[sl][:, idx, c0:],
                        ktp[p0:p1, hf, j * 128:(j + 1) * 128],
                        kqv[p0:p1, hf, n * CHW + c0:(n + 1) * CHW],
                        start=True, stop=True, skip_group_check=True)
            cur = []
            for sl in range(2):
                pt = ptp.tile([128, 2, CHW], bf16, name="pt")
                cur.append(pt)
                if c0s[0] == 0 and c0s[1] == 0:
                    nc.scalar.activation(out=pt, in_=sss[sl], func=AF.Exp)
                else:
                    for idx in range(2):
                        c0 = c0s[idx]
                        nc.scalar.activation(out=pt[:, idx, c0:],
                                             in_=sss[sl][:, idx, c0:],
                                             func=AF.Exp)
                for idx in range(2):
                    j = 2 * a + idx
                    if j >= 4 * n:
                        c0 = c0s[idx]
                        nc.gpsimd.tensor_tensor(
                            out=pt[:, idx, c0:c0 + 128],
                            in0=pt[:, idx, c0:c0 + 128],
                            in1=amask, op=OP.mult)
            pts[g] = cur

        pys_cur = [None]

        def pv_emit(g):
            n, hf, a = groups[g]
            jmax = 4 * n + 3
            cols = slice(n * CHW, (n + 1) * CHW)
            if hf == 0 and a == 0 and n >= 1:
                for o in range(8):
                    fillers.append(lambda pn=n - 1, o=o: proj_step(pn, o))
            if a == 0:
                pys_cur[0] = [ps.tile([128, CHW], f32, name=f"py{sl}",
                                      tag="py", bufs=2) for sl in range(2)]
            pys = pys_cur[0]
            if 2 * a + 1 >= 4 * n:
                drain_until(f"vt{n}")  # this group's v blocks must be placed
            c0s = [max(0, 128 * (2 * a + idx) - 512 * n) for idx in range(2)]
            for sl in range(2):
                for idx in range(2):
                    j = 2 * a + idx
                    c0 = c0s[idx]
                    nc.tensor.matmul(
                        pys[sl][:, c0:], v_stationary(j, hf, sl),
                        pts[g][sl][:, idx, c0:],
                        start=(j == 0), stop=(j == jmax),
                        skip_group_check=True)
            del pts[g]
            if a == 2 * n + 1:
                # ---- normalization for the head pair ----
                ri = rbp.tile([128, CHW], f32, name="ri")
                rb = rbp.tile([128, CHW], f32, name="rb")
                nc.vector.tensor_copy(ri[64:128, :], pys[0][64:128, :])
                nc.sync.dma_start(rb[0:64, :], ri[64:128, :])
                nc.vector.reciprocal_approx_fast(ri[0:64, :],
                                                 pys[1][0:64, :])
                nc.vector.reciprocal_approx_fast(rb[0:64, :], rb[0:64, :])
                nc.vector.tensor_tensor(out=ysb[0:64, hf, cols],
                                        in0=pys[0][0:64, :],
                                        in1=rb[0:64, :], op=OP.mult)
                nc.sync.dma_start(rb[64:128, :], ri[0:64, :])
                nc.vector.tensor_tensor(out=ysb[64:128, hf, cols],
                                        in0=pys[1][64:128, :],
                                        in1=rb[64:128, :], op=OP.mult)

        # ---- chunk-0 QKV k/q tiles, then the pipeline ----
        with nc.named_scope("qkv0"):
            for m in range(4):
                qkv_step(0, m)
        # warmup fillers: v tiles of chunk 0 (k/q already emitted above)
        fillers.append(("mark", "qkv0"))
        for m in (4, 5):
            fillers.append(lambda m=m: qkv_step(0, m))
        for j in range(4):
            for hf in range(2):
                fillers.append(lambda j=j, hf=hf: v_transpose_step(j, hf))
        fillers.append(("mark", "vt0"))

        HORIZON = 6
        with nc.named_scope("pipe"):
            for slot in range(NG + LOOKAHEAD):
                if slot >= LOOKAHEAD:
                    pv_emit(slot - LOOKAHEAD)
                nitems = sum(1 for f in fillers if not isinstance(f, tuple))
                rem = min(NG + LOOKAHEAD - slot, HORIZON)
                consume(max(0, (nitems + rem - 1) // rem))
                if slot < NG:
                    qk_emit(slot)
            while fillers:
                consume(1)

        # ---- tail: proj for chunk 3 in ss-shaped o-pairs ----
        with nc.named_scope("tail"):
            for op2 in range(4):
                acc = ps.tile([128, 2, CHW], f32, name="tacc", tag="ss",
                              bufs=2)
                for i in range(2):
                    o = 2 * op2 + i
                    for kk in range(2):
                        nc.tensor.matmul(
                            acc[:, i, :], wp[:, kk, o * 128:(o + 1) * 128],
                            ysb[:, kk, 3 * CHW:4 * CHW],
                            start=(kk == 0), stop=(kk == 1),
                            skip_group_check=True)
                ot = osp.tile([128, 2, CHW], bf16, name="ot2")
                for i in range(2):
                    o = 2 * op2 + i
                    if i == 0:
                        nc.vector.tensor_scalar_add(out=ot[:, i, :],
                                                    in0=acc[:, i, :],
                                                    scalar1=bp_s[:, o:o + 1])
                    else:  # ACT is idle at the tail — split the drains
                        nc.scalar.activation(out=ot[:, i, :],
                                             in_=acc[:, i, :],
                                             func=AF.Identity,
                                             bias=bp_s[:, o:o + 1])
                nc.sync.dma_start(
                    outp[op2 * 256:(op2 + 1) * 256,
                         3 * CHW:4 * CHW].rearrange("(a p) t -> p a t", a=2),
                    ot)

    nc.compile()
    return nc


def _host_inputs(x, W_kqv, b_kqv, W_proj, b_proj):
    import ml_dtypes
    bf16 = ml_dtypes.bfloat16

    x = np.asarray(x, dtype=np.float32)
    W_kqv = np.asarray(W_kqv, dtype=np.float32)
    b_kqv = np.asarray(b_kqv, dtype=np.float32)
    W_proj = np.asarray(W_proj, dtype=np.float32)
    b_proj = np.asarray(b_proj, dtype=np.float32)

    ss, tt = np.meshgrid(np.arange(128), np.arange(128), indexing="ij")
    amask = (ss <= tt).astype(bf16)  # 0/1 multiplicative mask
    ident = np.eye(128, dtype=bf16)

    xts = [np.ascontiguousarray(x[b].T.astype(bf16)) for b in range(B)]

    in_maps = []
    for c in range(NCORES):
        b, g = c // 4, c % 4
        heads = [4 * g + i for i in range(HPC)]
        wl = np.concatenate(
            [W_kqv[h * 192:h * 192 + 64] for h in heads]
            + [W_kqv[h * 192 + 64:h * 192 + 128] * 0.125 for h in heads]
            + [W_kqv[h * 192 + 128:h * 192 + 192] for h in heads], axis=0)
        bl = np.concatenate(
            [b_kqv[h * 192:h * 192 + 64] for h in heads]
            + [b_kqv[h * 192 + 64:h * 192 + 128] * 0.125 for h in heads]
            + [b_kqv[h * 192 + 128:h * 192 + 192] for h in heads])
        bpl = b_proj if g == 0 else np.zeros_like(b_proj)
        in_maps.append({
            "xt": xts[b],
            "wkqv": np.ascontiguousarray(wl.T.astype(bf16)),
            "bkq": np.ascontiguousarray(bl.reshape(6, 128).T),
            "wproj": np.ascontiguousarray(
                W_proj[:, 256 * g:256 * (g + 1)].T.astype(bf16)),
            "bp": np.ascontiguousarray(bpl.reshape(8, 128).T),
            "amask": amask,
            "ident": ident,
        })
    return in_maps


def kernel(x, W_kqv, b_kqv, W_proj, b_proj):
    from concourse.bass_utils import run_bass_kernel_spmd

    if "nc" not in _cache:
        _cache["nc"] = _build_nc()
    nc = _cache["nc"]

    in_maps = _host_inputs(x, W_kqv, b_kqv, W_proj, b_proj)
    trace = bool(int(os.environ.get("KERNEL_TRACE", "0")))
    r = run_bass_kernel_spmd(nc, in_maps, core_ids=list(range(NCORES)),
                             trace=trace)
    if trace:
        _cache["last_results"] = r
        print(f"HW exec time: {r.exec_time_ns} ns")

    out = np.empty((B, T, C), dtype=np.float32)
    for b in range(B):
        acc = np.zeros((C, T), dtype=np.float32)
        for g in range(4):
            acc += np.asarray(r.results[4 * b + g]["outp"],
                              dtype=np.float32)
        out[b] = acc.T
    return out


# revision 55
# speedup vs baseline: 1.1997x; 1.0083x over previous
"""Causal self-attention (B=2, T=2048, C=1024, nh=16) on 8 TRN2 NeuronCores.

Sharding: core c = 4*b + g handles batch b (2048 tokens) and head-group g
(4 heads).  Megatron-style: QKV rows and proj columns sharded by head group;
the proj partial sums are reduced on the host (the "all-reduce").

Per-core kernel, all matmul operands bf16 (same PE rate as fp32r but FWL
weight loads + half SBUF/DMA bandwidth), PSUM accumulation fp32:
  1. QKV projection kqvT[f,t] = Wl @ x_b.T -> ktp (k, head-pair packed on
     partitions: even head 0:64, odd head 64:128), q and v slots in kqv.
  2. v tiles PE-transposed to [s,d] layout, packed next to shared all-ones
     column blocks so the PV matmul also emits softmax row-sums broadcast
     across the complement 64 partitions.
  3. QK row-tiled: per j-tile the two heads of a pair run CONCURRENTLY in
     PE row groups (0,0)/(64,0) with 64-partition operands (no zero pad).
     exp on ACT -> pt bf16 (parked in SBUF), multiplicative 0/1 mask on
     diagonal blocks (gpsimd), PV accumulation v_aug.T @ P.T -> yT+rowsum.
  4. Software pipeline: the QK+exp stream runs LOOKAHEAD groups ahead of
     the PV stream so ACT's quadratic exp load is prepaid early; QKV(n+1),
     v transposes and proj(n-1) matmuls braid into PE slack.
  5. normalize: fast-approx reciprocal off PSUM rowsums + DMA partition
     shift; multiplies read yT straight from PSUM. proj partials on PE,
     bf16 partial outputs summed on host.
"""

import os
import numpy as np

B, T, C, NH, HD = 2, 2048, 1024, 16, 64
HPC = 4  # heads per core
NCORES = 8
NCH = 4       # 512-wide t-chunks
CHW = 512
LOOKAHEAD = 12

_cache = {}


def _build_nc():
    from contextlib import ExitStack

    import concourse.bass as bass
    import concourse.tile as tile
    from concourse import bacc, mybir

    f32 = mybir.dt.float32
    bf16 = mybir.dt.bfloat16
    f8 = mybir.dt.float8e4
    AF = mybir.ActivationFunctionType
    OP = mybir.AluOpType

    nc = bacc.Bacc("TRN2", target_bir_lowering=False, debug=False,
                   num_devices=NCORES)

    xt = nc.dram_tensor("xt", [C, T], bf16, kind="ExternalInput").ap()
    wkqv = nc.dram_tensor("wkqv", [C, 3 * HPC * HD], bf16,
                          kind="ExternalInput").ap()
    bkq = nc.dram_tensor("bkq", [128, 6], f32, kind="ExternalInput").ap()
    wproj = nc.dram_tensor("wproj", [HPC * HD, C], bf16,
                           kind="ExternalInput").ap()
    bp = nc.dram_tensor("bp", [128, 8], f32, kind="ExternalInput").ap()
    amask_d = nc.dram_tensor("amask", [128, 128], bf16,
                             kind="ExternalInput").ap()
    ident_d = nc.dram_tensor("ident", [128, 128], bf16,
                             kind="ExternalInput").ap()
    outp = nc.dram_tensor("outp", [C, T], bf16, kind="ExternalOutput").ap()

    with tile.TileContext(nc) as tc, ExitStack() as ctx:
        sing = ctx.enter_context(tc.tile_pool(name="sing", bufs=1))
        xpool = ctx.enter_context(tc.tile_pool(name="xpool", bufs=3))
        ptp = ctx.enter_context(tc.tile_pool(name="ptp",
                                             bufs=2 * (LOOKAHEAD + 2)))
        rbp = ctx.enter_context(tc.tile_pool(name="rbp", bufs=4))
        osp = ctx.enter_context(tc.tile_pool(name="osp", bufs=3))
        ps = ctx.enter_context(tc.tile_pool(name="ps", bufs=2, space="PSUM"))

        # ---- resident SBUF tensors ----
        wk = sing.tile([128, 8, 768], bf16, name="wk")
        ktp = sing.tile([128, 2, T], bf16, name="ktp")
        kqv = sing.tile([128, 4, T], bf16, name="kqv")
        vsb = sing.tile([128, 32, 256], bf16, name="vsb")
        ysb = sing.tile([128, 2, T], bf16, name="ysb")
        wp = sing.tile([128, 2, C], bf16, name="wp")
        bkq_s = sing.tile([128, 6], f32, name="bkq_s")
        bp_s = sing.tile([128, 8], f32, name="bp_s")
        amask = sing.tile([128, 128], bf16, name="amask")
        ident = sing.tile([128, 128], bf16, name="ident")

        nc.vector.memset(vsb[:, :, 64:192], 1.0)

        xt_r = xt.rearrange("(kk p) t -> p kk t", p=128)
        wkqv_r = wkqv.rearrange("(kk p) f -> p kk f", p=128)

        xts_tiles = {}

        def prefetch_x(n):
            nxt = xpool.tile([128, 8, CHW], bf16, name="xts")
            for k in range(8):
                eng = nc.scalar if k % 2 else nc.sync
                eng.dma_start(nxt[:, k, :], xt_r[:, k, n * CHW:(n + 1) * CHW])
            xts_tiles[n] = nxt

        # initial loads: interleave weights and first x chunk per k-tile so
        # the first QKV matmuls start as soon as possible
        xts0 = xpool.tile([128, 8, CHW], bf16, name="xts")
        xts_tiles[0] = xts0
        for k in range(8):
            e1, e2 = (nc.scalar, nc.sync) if k % 2 else (nc.sync, nc.scalar)
            e1.dma_start(wk[:, k, :], wkqv_r[:, k, :])
            e2.dma_start(xts0[:, k, :], xt_r[:, k, 0:CHW])
        nc.sync.dma_start(bkq_s, bkq)
        nc.scalar.dma_start(ident, ident_d)
        nc.scalar.dma_start(amask, amask_d)
        nc.sync.dma_start(wp, wproj.rearrange("(kk p) f -> p kk f", p=128))
        nc.sync.dma_start(bp_s, bp)

        def v_stationary(j, hf, sl):
            off = (j * 2 + hf) * 256 + 128 * sl
            return vsb.rearrange("p a b -> p (a b)")[:, off:off + 128]

        def qkv_step(n, m):
            """one full m-tile: 8 contraction matmuls + drain (atomic so the
            shared acc tag never holds a live accumulator across items)"""
            cols = slice(n * CHW, (n + 1) * CHW)
            acc = ps.tile([128, CHW], f32, name="acc", tag="acc", bufs=2)
            for k in range(8):
                nc.tensor.matmul(acc, wk[:, k, m * 128:(m + 1) * 128],
                                 xts_tiles[n][:, k, :],
                                 start=(k == 0), stop=(k == 7),
                                 skip_group_check=True)
            dst = ktp[:, m, cols] if m < 2 else kqv[:, m - 2, cols]
            nc.vector.tensor_scalar_add(out=dst, in0=acc,
                                        scalar1=bkq_s[:, m:m + 1])

        def v_transpose_step(j, hf):
            tpf = ps.tile([128, CHW], f32, name="tp", tag="acc", bufs=2)
            tp = tpf.bitcast(bf16)[:, 0:128]
            nc.tensor.transpose(tp, kqv[:, 2 + hf, j * 128:(j + 1) * 128],
                                ident)
            base = vsb.rearrange("p a b -> p (a b)")
            anch = base[:, (j * 2 + hf) * 256:(j * 2 + hf) * 256 + 1]
            dst = bass.AP(tensor=anch.tensor, offset=anch.offset,
                          ap=[anch.ap[0], [192, 2], [1, 64]])
            nc.vector.tensor_copy(dst, tp.rearrange("p (a b) -> p a b", a=2))

        def proj_step(pn, o):
            acc = ps.tile([128, CHW], f32, name="pacc", tag="acc", bufs=2)
            for kk in range(2):
                nc.tensor.matmul(acc, wp[:, kk, o * 128:(o + 1) * 128],
                                 ysb[:, kk, pn * CHW:(pn + 1) * CHW],
                                 start=(kk == 0), stop=(kk == 1),
                                 skip_group_check=True)
            ot = osp.tile([128, CHW], bf16, name="ot")
            nc.vector.tensor_scalar_add(out=ot, in0=acc,
                                        scalar1=bp_s[:, o:o + 1])
            nc.sync.dma_start(
                outp[o * 128:(o + 1) * 128, pn * CHW:(pn + 1) * CHW], ot)

        # ---------------- software pipeline ----------------
        groups = [(n, hf, a)
                  for n in range(NCH) for hf in range(2)
                  for a in range(2 * n + 2)]
        NG = len(groups)

        # filler queue with named barriers: ("mark", name) entries
        fillers = []
        marks_done = set()

        def drain_until(name):
            while name not in marks_done:
                item = fillers.pop(0)
                if isinstance(item, tuple):
                    marks_done.add(item[1])
                else:
                    item()

        def consume(k):
            while k > 0 and fillers:
                item = fillers.pop(0)
                if isinstance(item, tuple):
                    marks_done.add(item[1])
                else:
                    item()
                    k -= 1

        def add_qk_chunk_fillers(n):
            """called when the QK stream enters chunk n: queue next chunk's
            QKV matmuls and v transposes"""
            if n + 1 < NCH:
                fillers.append(lambda m=n + 1: prefetch_x(m))
                for m in range(6):
                    fillers.append(lambda m=m, nn=n + 1: qkv_step(nn, m))
                fillers.append(("mark", f"qkv{n + 1}"))
                for j in range(4 * (n + 1), 4 * (n + 1) + 4):
                    for hf in range(2):
                        fillers.append(
                            lambda j=j, hf=hf: v_transpose_step(j, hf))
                fillers.append(("mark", f"vt{n + 1}"))

        pts = {}

        def qk_emit(g):
            n, hf, a = groups[g]
            if a == 0 and hf == 0:
                drain_until(f"qkv{n}")  # ktp/q for chunk n must be emitted
                add_qk_chunk_fillers(n)
            c0s = [max(0, 128 * (2 * a + idx) - 512 * n) for idx in range(2)]
            sss = [ps.tile([128, 2, CHW], f32, name=f"ss{sl}", tag="ss",
                           bufs=2) for sl in range(2)]
            for idx in range(2):
                j = 2 * a + idx
                c0 = c0s[idx]
                for sl in range(2):
                    p0, p1 = 64 * sl, 64 * (sl + 1)
                    nc.tensor.matmul(
                        sss[sl][:, idx, c0:],
                        ktp[p0:p1, hf, j * 128:(j + 1) * 128],
                        kqv[p0:p1, hf, n * CHW + c0:(n + 1) * CHW],
                        start=True, stop=True, skip_group_check=True)
            cur = []
            for sl in range(2):
                pt = ptp.tile([128, 2, CHW], bf16, name="pt")
                cur.append(pt)
                if c0s[0] == 0 and c0s[1] == 0:
                    nc.scalar.activation(out=pt, in_=sss[sl], func=AF.Exp)
                else:
                    for idx in range(2):
                        c0 = c0s[idx]
                        nc.scalar.activation(out=pt[:, idx, c0:],
                                             in_=sss[sl][:, idx, c0:],
                                             func=AF.Exp)
                for idx in range(2):
                    j = 2 * a + idx
                    if j >= 4 * n:
                        c0 = c0s[idx]
                        nc.gpsimd.tensor_tensor(
                            out=pt[:, idx, c0:c0 + 128],
                            in0=pt[:, idx, c0:c0 + 128],
                            in1=amask, op=OP.mult)
            pts[g] = cur

        pys_cur = [None]

        def pv_emit(g):
            n, hf, a = groups[g]
            jmax = 4 * n + 3
            cols = slice(n * CHW, (n + 1) * CHW)
            if hf == 0 and a == 0 and n >= 1:
                for o in range(8):
                    fillers.append(lambda pn=n - 1, o=o: proj_step(pn, o))
            if a == 0:
                pys_cur[0] = [ps.tile([128, CHW], f32, name=f"py{sl}",
                                      tag="py", bufs=2) for sl in range(2)]
            pys = pys_cur[0]
            if 2 * a + 1 >= 4 * n:
                drain_until(f"vt{n}")  # this group's v blocks must be placed
            c0s = [max(0, 128 * (2 * a + idx) - 512 * n) for idx in range(2)]
            for sl in range(2):
                for idx in range(2):
                    j = 2 * a + idx
                    c0 = c0s[idx]
                    nc.tensor.matmul(
                        pys[sl][:, c0:], v_stationary(j, hf, sl),
                        pts[g][sl][:, idx, c0:],
                        start=(j == 0), stop=(j == jmax),
                        skip_group_check=True)
            del pts[g]
            if a == 2 * n + 1:
                # ---- normalization for the head pair ----
                ri = rbp.tile([128, CHW], f32, name="ri")
                rb = rbp.tile([128, CHW], f32, name="rb")
                nc.vector.tensor_copy(ri[64:128, :], pys[0][64:128, :])
                nc.sync.dma_start(rb[0:64, :], ri[64:128, :])
                nc.vector.reciprocal_approx_fast(ri[0:64, :],
                                                 pys[1][0:64, :])
                nc.vector.reciprocal_approx_fast(rb[0:64, :], rb[0:64, :])
                nc.vector.tensor_tensor(out=ysb[0:64, hf, cols],
                                        in0=pys[0][0:64, :],
                                        in1=rb[0:64, :], op=OP.mult)
                nc.sync.dma_start(rb[64:128, :], ri[0:64, :])
                nc.vector.tensor_tensor(out=ysb[64:128, hf, cols],
                                        in0=pys[1][64:128, :],
                                        in1=rb[64:128, :], op=OP.mult)

        # ---- chunk-0 QKV k/q tiles, then the pipeline ----
        with nc.named_scope("qkv0"):
            for m in range(4):
                qkv_step(0, m)
        # warmup fillers: v tiles of chunk 0 (k/q already emitted above)
        fillers.append(("mark", "qkv0"))
        for m in (4, 5):
            fillers.append(lambda m=m: qkv_step(0, m))
        for j in range(4):
            for hf in range(2):
                fillers.append(lambda j=j, hf=hf: v_transpose_step(j, hf))
        fillers.append(("mark", "vt0"))

        HORIZON = 6
        with nc.named_scope("pipe"):
            for slot in range(NG + LOOKAHEAD):
                if slot >= LOOKAHEAD:
                    pv_emit(slot - LOOKAHEAD)
                nitems = sum(1 for f in fillers if not isinstance(f, tuple))
                rem = min(NG + LOOKAHEAD - slot, HORIZON)
                consume(max(0, (nitems + rem - 1) // rem))
                if slot < NG:
                    qk_emit(slot)
            while fillers:
                consume(1)

        # ---- tail: proj for chunk 3 in ss-shaped o-pairs ----
        with nc.named_scope("tail"):
            for op2 in range(4):
                acc = ps.tile([128, 2, CHW], f32, name="tacc", tag="ss",
                              bufs=2)
                for i in range(2):
                    o = 2 * op2 + i
                    for kk in range(2):
                        nc.tensor.matmul(
                            acc[:, i, :], wp[:, kk, o * 128:(o + 1) * 128],
                            ysb[:, kk, 3 * CHW:4 * CHW],
                            start=(kk == 0), stop=(kk == 1),
                            skip_group_check=True)
                ot = osp.tile([128, 2, CHW], bf16, name="ot2")
                for i in range(2):
                    o = 2 * op2 + i
                    if i == 0:
                        nc.vector.tensor_scalar_add(out=ot[:, i, :],
                                                    in0=acc[:, i, :],
                                                    scalar1=bp_s[:, o:o + 1])
                    else:  # ACT is idle at the tail — split the drains
                        nc.scalar.activation(out=ot[:, i, :],
                                             in_=acc[:, i, :],
                                             func=AF.Identity,
                                             bias=bp_s[:, o:o + 1])
                nc.sync.dma_start(
                    outp[op2 * 256:(op2 + 1) * 256,
                         3 * CHW:4 * CHW].rearrange("(a p) t -> p a t", a=2),
                    ot)

    nc.compile()
    return nc


def _host_inputs(x, W_kqv, b_kqv, W_proj, b_proj):
    import ml_dtypes
    bf16 = ml_dtypes.bfloat16

    x = np.asarray(x, dtype=np.float32)
    W_kqv = np.asarray(W_kqv, dtype=np.float32)
    b_kqv = np.asarray(b_kqv, dtype=np.float32)
    W_proj = np.asarray(W_proj, dtype=np.float32)
    b_proj = np.asarray(b_proj, dtype=np.float32)

    ss, tt = np.meshgrid(np.arange(128), np.arange(128), indexing="ij")
    amask = (ss <= tt).astype(bf16)  # 0/1 multiplicative mask
    ident = np.eye(128, dtype=bf16)

    xts = [np.ascontiguousarray(x[b].T.astype(bf16)) for b in range(B)]

    in_maps = []
    for c in range(NCORES):
        b, g = c // 4, c % 4
        heads = [4 * g + i for i in range(HPC)]
        wl = np.concatenate(
            [W_kqv[h * 192:h * 192 + 64] for h in heads]
            + [W_kqv[h * 192 + 64:h * 192 + 128] * 0.125 for h in heads]
            + [W_kqv[h * 192 + 128:h * 192 + 192] for h in heads], axis=0)
        bl = np.concatenate(
            [b_kqv[h * 192:h * 192 + 64] for h in heads]
            + [b_kqv[h * 192 + 64:h * 192 + 128] * 0.125 for h in heads]
            + [b_kqv[h * 192 + 128:h * 192 + 192] for h in heads])
        bpl = b_proj if g == 0 else np.zeros_like(b_proj)
        in_maps.append({
            "xt": xts[b],
            "wkqv": np.ascontiguousarray(wl.T.astype(bf16)),
            "bkq": np.ascontiguousarray(bl.reshape(6, 128).T),
            "wproj": np.ascontiguousarray(
                W_proj[:, 256 * g:256 * (g + 1)].T.astype(bf16)),
            "bp": np.ascontiguousarray(bpl.reshape(8, 128).T),
            "amask": amask,
            "ident": ident,
        })
    return in_maps


def kernel(x, W_kqv, b_kqv, W_proj, b_proj):
    from concourse.bass_utils import run_bass_kernel_spmd

    if "nc" not in _cache:
        _cache["nc"] = _build_nc()
    nc = _cache["nc"]

    in_maps = _host_inputs(x, W_kqv, b_kqv, W_proj, b_proj)
    trace = bool(int(os.environ.get("KERNEL_TRACE", "0")))
    r = run_bass_kernel_spmd(nc, in_maps, core_ids=list(range(NCORES)),
                             trace=trace)
    if trace:
        _cache["last_results"] = r
        print(f"HW exec time: {r.exec_time_ns} ns")

    out = np.empty((B, T, C), dtype=np.float32)
    for b in range(B):
        acc = np.zeros((C, T), dtype=np.float32)
        for g in range(4):
            acc += np.asarray(r.results[4 * b + g]["outp"],
                              dtype=np.float32)
        out[b] = acc.T
    return out
